# revision 22
# baseline (speedup 1.0000x reference)
"""CrossAttention kernel for 8 Trainium2 NeuronCores (Bass/Tile).

Sharding: tensor-parallel over heads. Core i handles heads {2i, 2i+1} for
both batch elements. LayerNorm scale/bias and the q-scale are folded into
the projection weights on the host. Projections run on RAW (un-normalized)
tokens: the per-token mean correction rides the matmul as a K=1 row and
the 1/sigma scaling is applied by one scalar_tensor_tensor per tile.
LN sum-of-squares uses fp8 squares (Act Square) contracted with DoubleRow
matmuls at half cost. exp(alibi) is precomputed on the host (bf16):
softmax weights are exp(scores) * exp(alibi), so the alibi add becomes a
2-byte-fast-path multiply (DVE/Pool) after the exp instead of an f32 PSUM
add. The softmax denominator rides the AV matmul as a ones-column of V.
The q-projections are software-pipelined into the attention phase (one
token tile per query tile). Output partials are written bf16; host gather
sums the 8 partial [dout, tok] projections.
"""

import os
import sys

for _p in ("/opt/trn_rl_repo", "/root/.axon_site/_ro/trn_rl_repo"):
    if os.path.isdir(_p) and _p not in sys.path:
        sys.path.insert(0, _p)

import numpy as np
import ml_dtypes

import concourse.bass as bass
import concourse.tile as tile
from concourse import bacc, mybir
from concourse.masks import make_identity

BF16 = ml_dtypes.bfloat16

HEADS = 16
N_CORES = 8
H_PER_CORE = HEADS // N_CORES  # 2
DH = 64
LN_EPS = 1e-5

B = 2
N_TOK = 2048
D = 1024

QT = 512            # query tile (free dim of scores matmuls)
KT = 128            # key tile (partition dim of scoresT)
TT = 512            # token tile for LN/projection phase
N_DT = D // 128     # 8 contraction tiles of 128 over d

POOL_MUL_MOD = 8    # kt % POOL_MUL_MOD < POOL_MUL_CNT -> eal-mult on Pool
POOL_MUL_CNT = 0


def build_program(n_tok=N_TOK, with_pbias=False):
    """Build the single-core SPMD Bass program. Returns nc."""
    nc = bacc.Bacc("TRN2")
    f32 = mybir.dt.float32
    f32r = mybir.dt.float32r
    bf16 = mybir.dt.bfloat16
    fp8 = mybir.dt.float8e4

    n_tt = n_tok // TT          # token tiles per batch
    n_qt = n_tok // QT          # query tiles per batch
    n_kt = n_tok // KT          # key tiles per batch
    assert n_tt == n_qt  # q-proj pipelined per query tile

    # ---- DRAM parameters (per-core shards, host-prepped) ----
    xT = nc.declare_dram_parameter("xT", [B, D, n_tok], bf16, isOutput=False)
    cT = nc.declare_dram_parameter("cT", [B, D, n_tok], bf16, isOutput=False)
    # exp(alibi), transposed: [h, key, q]
    ealT = nc.declare_dram_parameter(
        "ealT", [H_PER_CORE, n_tok, n_tok], bf16, isOutput=False)
    wqT = nc.declare_dram_parameter("wqT", [D, 128], bf16, isOutput=False)
    wkT = nc.declare_dram_parameter("wkT", [D, 128], bf16, isOutput=False)
    wvT = nc.declare_dram_parameter("wvT", [D, 128], bf16, isOutput=False)
    # rows: -wbar_q, -wbar_k, -wbar_v   (lhsT for the K=1 mu-correction row)
    wbar = nc.declare_dram_parameter("wbar", [3, 128], bf16, isOutput=False)
    woT = nc.declare_dram_parameter("woT", [128, D], bf16, isOutput=False)
    if with_pbias:
        # columns: q/k/v projection bias (ln_b folded through W), bf16 lhsT
        pbias = nc.declare_dram_parameter("pbias", [3, 128], bf16,
                                          isOutput=False)
    bo_r = nc.declare_dram_parameter("bo_r", [128, N_DT], f32, isOutput=False)

    outT = nc.declare_dram_parameter(
        "outT", [D, B * n_tok], bf16, isOutput=True)

    xT_r = xT.rearrange("b (dt p) n -> b p dt n", p=128)
    cT_r = cT.rearrange("b (dt p) n -> b p dt n", p=128)
    woT_r = woT.rearrange("c (dt n) -> c dt n", n=128)

    with tile.TileContext(nc) as tc:
        with tc.tile_pool(name="const", bufs=1) as const_pool:
            ident = const_pool.tile([128, 128], bf16)
            make_identity(nc, ident)
            zeros128 = const_pool.tile([128, 1], f32)
            nc.vector.memset(zeros128, 0.0)
            eps4 = const_pool.tile([4, 1], f32)
            nc.vector.memset(eps4, LN_EPS)
            # stats lhsT: onehot[:, u, j] is all-ones iff j == u
            onehot = const_pool.tile([128, n_tt, 4], bf16)
            nc.vector.memset(onehot, 0.0)
            # fp8 onehot pairs for the DoubleRow sxx matmuls
            # (DoubleRow ldweights requires stationary free dim >= 16)
            onehot8 = const_pool.tile([128, n_tt, 2, 16], fp8)
            nc.vector.memset(onehot8, 0.0)
            for u in range(n_tt):
                nc.vector.memset(onehot[:, u, u:u + 1], 1.0)
                nc.vector.memset(onehot8[:, u, :, u:u + 1], 1.0)

            wq_sb = const_pool.tile([128, N_DT, 128], bf16)
            wk_sb = const_pool.tile([128, N_DT, 128], bf16)
            wv_sb = const_pool.tile([128, N_DT, 128], bf16)
            nc.sync.dma_start(out=wq_sb, in_=wqT.rearrange("(dt p) c -> p dt c", p=128))
            nc.sync.dma_start(out=wk_sb, in_=wkT.rearrange("(dt p) c -> p dt c", p=128))
            nc.sync.dma_start(out=wv_sb, in_=wvT.rearrange("(dt p) c -> p dt c", p=128))
            wbar_sb = const_pool.tile([1, 3, 128], bf16)
            nc.sync.dma_start(out=wbar_sb, in_=wbar[None, :, :])
            wo_sb = const_pool.tile([128, N_DT, 128], bf16)
            nc.sync.dma_start(out=wo_sb, in_=woT_r)
            if with_pbias:
                pbias_sb = const_pool.tile([1, 3, 128], bf16)
                nc.sync.dma_start(out=pbias_sb, in_=pbias[None, :, :])
            bo_sb = const_pool.tile([128, N_DT], f32)
            nc.sync.dma_start(out=bo_sb, in_=bo_r[:, :])

            # persistent activations: [c(128), b, tok]
            qT_sb = const_pool.tile([128, B, n_tok], f32r)
            kT_sb = const_pool.tile([128, B, n_tok], f32r)
            # v natural (+ones col): [key(128), b*n_kt*h, 66]
            vaug_sb = const_pool.tile([128, B * n_kt * H_PER_CORE, 66], bf16)
            nc.vector.memset(vaug_sb[:, :, 64:65], 1.0)

            def vaug_idx(b, kt, h):
                return (b * n_kt + kt) * H_PER_CORE + h

            # SBUF pools that live for the whole program (x raws are read
            # during the attention phase by the pipelined q-projections)
            with tc.tile_pool(name="raw_p", bufs=2 * n_tt + 1) as raw_p, \
                 tc.tile_pool(name="sq8_p", bufs=2) as sq8_p, \
                 tc.tile_pool(name="pha", bufs=2) as pha, \
                 tc.tile_pool(name="stat_tmp", bufs=1) as stat_tmp, \
                 tc.tile_pool(name="stat_sb", bufs=2) as stat_sb:

                def ln_stats(raws):
                    """DMA'd raw tiles -> (m_row, inv_row[, s_row]) rows."""
                    sx = stat_ps.tile([4, TT], f32, tag="sx")
                    sxx16 = stat_ps.tile([16, TT], f32, tag="sxx")
                    sxx = sxx16[0:4, :]
                    for u in range(n_tt):
                        sq8 = sq8_p.tile([128, N_DT, TT], fp8, tag="sq8")
                        nc.scalar.activation(
                            out=sq8, in_=raws[u],
                            func=mybir.ActivationFunctionType.Square,
                            bias=zeros128[:, 0:1], scale=1.0)
                        for dt in range(N_DT):
                            nc.tensor.matmul(
                                sx, onehot[:, u, :], raws[u][:, dt, :],
                                start=(u == 0 and dt == 0),
                                stop=(u == n_tt - 1 and dt == N_DT - 1))
                        for dp in range(N_DT // 2):
                            nc.tensor.matmul(
                                sxx16, onehot8[:, u, :, :],
                                sq8[:, dp * 2:dp * 2 + 2, :],
                                start=(u == 0 and dp == 0),
                                stop=(u == n_tt - 1 and dp == N_DT // 2 - 1),
                                perf_mode=mybir.MatmulPerfMode.DoubleRow)
                    # batched LN math on [n_tt, TT] rows
                    e = stat_tmp.tile([4, TT], f32, tag="e")
                    nc.vector.tensor_scalar_mul(e, sx, 1.0 / D)
                    ee = stat_tmp.tile([4, TT], f32, tag="ee")
                    nc.vector.tensor_mul(ee, e, e)
                    var = stat_tmp.tile([4, TT], f32, tag="var")
                    # var*D = Sxx - D*ee
                    nc.vector.scalar_tensor_tensor(
                        out=var, in0=ee, scalar=float(-D), in1=sxx,
                        op0=mybir.AluOpType.mult, op1=mybir.AluOpType.add)
                    sig0 = stat_tmp.tile([4, TT], f32, tag="sig0")
                    nc.scalar.activation(
                        out=sig0, in_=var,
                        func=mybir.ActivationFunctionType.Sqrt,
                        bias=eps4[:, 0:1], scale=1.0 / D)
                    invs = stat_tmp.tile([4, TT], f32, tag="invs")
                    nc.vector.reciprocal(invs, sig0)
                    invs_bf = stat_tmp.tile([4, TT], bf16, tag="invs_bf")
                    nc.vector.tensor_copy(invs_bf, invs)
                    m_bf = stat_tmp.tile([4, TT], bf16, tag="m_bf")
                    nc.vector.tensor_copy(m_bf, e)
                    # restage rows at partition 0 (matmul rhs and
                    # partition_broadcast both need base partition 0)
                    m_row = stat_sb.tile([1, n_tt, TT], bf16, tag="m_row")
                    inv_row = stat_sb.tile([1, n_tt, TT], bf16, tag="inv_row")
                    for u in range(n_tt):
                        nc.sync.dma_start(
                            out=m_row[:, u, :], in_=m_bf[u:u + 1, :])
                        nc.sync.dma_start(
                            out=inv_row[:, u, :], in_=invs_bf[u:u + 1, :])
                    if with_pbias:
                        # sigma row: pbias must not be scaled by 1/sigma
                        sig_bf = stat_tmp.tile([4, TT], bf16, tag="sig_bf")
                        nc.vector.tensor_copy(sig_bf, sig0)
                        s_row = stat_sb.tile([1, n_tt, TT], bf16, tag="s_row")
                        for u in range(n_tt):
                            nc.sync.dma_start(
                                out=s_row[:, u, :], in_=sig_bf[u:u + 1, :])
                    else:
                        s_row = None
                    return m_row, inv_row, s_row

                def proj_matmuls(ps, u, raw, wi, w_sb, stats):
                    m_row, _, s_row = stats
                    for dt in range(N_DT):
                        nc.tensor.matmul(
                            ps, w_sb[:, dt, :], raw[:, dt, :],
                            start=(dt == 0), stop=False)
                    nc.tensor.matmul(
                        ps, wbar_sb[:, wi, :], m_row[:, u, :],
                        start=False, stop=not with_pbias)
                    if with_pbias:
                        nc.tensor.matmul(
                            ps, pbias_sb[:, wi, :], s_row[:, u, :],
                            start=False, stop=True)

                def make_isb(u, stats):
                    isb = pha.tile([128, TT], bf16, tag="isb")
                    nc.gpsimd.partition_broadcast(isb, stats[1][:, u, :])
                    return isb

                def post_scale(dst, b, u, ps, isb):
                    nc.vector.scalar_tensor_tensor(
                        out=dst[:, b, u * TT:(u + 1) * TT],
                        in0=ps, scalar=1.0, in1=isb,
                        op0=mybir.AluOpType.mult, op1=mybir.AluOpType.mult)

                # ====== Phase A: context stats + k/v projections =========
                with tc.tile_pool(name="vtp", bufs=1) as vtp, \
                     tc.tile_pool(name="pha_ps", bufs=2, space="PSUM") as pha_ps, \
                     tc.tile_pool(name="stat_ps", bufs=2, space="PSUM") as stat_ps, \
                     tc.tile_pool(name="vt_ps", bufs=2, space="PSUM") as vt_ps:
                    vT_sb = vtp.tile([128, B, n_tok], bf16)
                    for b in range(B):
                        raws = []
                        for u in range(n_tt):
                            raw = raw_p.tile([128, N_DT, TT], bf16, tag="raw")
                            raws.append(raw)
                            nc.sync.dma_start(
                                out=raw,
                                in_=cT_r[b, :, :, u * TT:(u + 1) * TT])
                        stats = ln_stats(raws)
                        for u in range(n_tt):
                            isb = make_isb(u, stats)
                            for wi, w_sb, dst in ((1, wk_sb, kT_sb),
                                                  (2, wv_sb, vT_sb)):
                                ps = pha_ps.tile([128, TT], f32, tag="proj")
                                proj_matmuls(ps, u, raws[u], wi, w_sb, stats)
                                post_scale(dst, b, u, ps, isb)
                        # v natural (transpose vT) once per ctx batch
                        for kt in range(n_kt):
                            vt = vt_ps.tile([128, 128], bf16, tag="vt")
                            nc.tensor.transpose(
                                vt, vT_sb[:, b, kt * KT:(kt + 1) * KT], ident)
                            # both heads in one copy (adjacent vaug rows)
                            nc.vector.tensor_copy(
                                vaug_sb[:, vaug_idx(b, kt, 0):
                                        vaug_idx(b, kt, 0) + 2, 0:64],
                                vt.rearrange("p (h c) -> p h c", h=2))

                    # x raws + stats (q projections are pipelined into
                    # the attention phase below)
                    x_raws = []
                    for b in range(B):
                        raws = []
                        for u in range(n_tt):
                            raw = raw_p.tile([128, N_DT, TT], bf16, tag="raw")
                            raws.append(raw)
                            nc.sync.dma_start(
                                out=raw,
                                in_=xT_r[b, :, :, u * TT:(u + 1) * TT])
                        x_raws.append(raws)
                    x_stats = [ln_stats(x_raws[b]) for b in range(B)]

                # ====== Phase B: attention + output projection ===========
                with tc.tile_pool(name="ealp", bufs=2) as ealp, \
                     tc.tile_pool(name="exp_sb", bufs=3) as exp_sb, \
                     tc.tile_pool(name="exm_sb", bufs=5) as exm_sb, \
                     tc.tile_pool(name="phbd", bufs=1) as phbd, \
                     tc.tile_pool(name="phfo", bufs=2) as phfo, \
                     tc.tile_pool(name="sc2_ps", bufs=2, space="PSUM") as sc2_ps, \
                     tc.tile_pool(name="av_ps", bufs=1, space="PSUM") as av_ps:
                    def make_tail(qt, av):
                        q0 = qt * QT

                        def part0():
                            for b in range(B):
                                den = phbd.tile([1, 2, QT], f32, tag="den",
                                                name=f"den{b}")
                                for h in range(H_PER_CORE):
                                    nc.vector.tensor_copy(
                                        den[:, h, :], av[b][h][64:65, :])
                                rden = phbd.tile([1, 2, QT], f32, tag="rden",
                                                 name=f"rden{b}")
                                nc.vector.reciprocal_approx_fast(rden, den)
                                for h in range(H_PER_CORE):
                                    rb = phbd.tile([64, QT], f32,
                                                   tag=f"rb{b}{h}")
                                    nc.gpsimd.partition_broadcast(
                                        rb, rden[:, h, :])
                                    st[f"rb{b}{h}"] = rb

                        def part1():
                            for b in range(B):
                                o_sb = phbd.tile([128, QT], bf16,
                                                 tag=f"o_sb{b}")
                                for h in range(H_PER_CORE):
                                    nc.vector.tensor_mul(
                                        o_sb[h * 64:(h + 1) * 64, :],
                                        av[b][h][0:64, :], st[f"rb{b}{h}"])
                                st[f"o_sb{b}"] = o_sb

                        def part2(b):
                            o_sb = st[f"o_sb{b}"]
                            outT_r = outT.rearrange("(dt p) n -> p dt n",
                                                    p=128)
                            for dp in range(N_DT // 2):
                                fp2 = sc2_ps.tile([128, 2, QT], f32,
                                                  tag="sc2", name="fp2")
                                fo = phfo.tile([128, 2, QT], bf16, tag="fo")
                                for j in range(2):
                                    dt = dp * 2 + j
                                    nc.tensor.matmul(
                                        fp2[:, j, :], wo_sb[:, dt, :], o_sb,
                                        start=True, stop=True)
                                    if j == 0:
                                        nc.scalar.activation(
                                            out=fo[:, j, :], in_=fp2[:, j, :],
                                            func=mybir.ActivationFunctionType
                                            .Identity,
                                            bias=bo_sb[:, dt:dt + 1],
                                            scale=1.0)
                                    else:
                                        nc.vector.tensor_scalar_add(
                                            fo[:, j, :], fp2[:, j, :],
                                            bo_sb[:, dt:dt + 1])
                                nc.sync.dma_start(
                                    out=outT_r[
                                        :, dp * 2:dp * 2 + 2,
                                        b * n_tok + q0:b * n_tok + q0 + QT],
                                    in_=fo)

                        st = {}
                        return [part0, part1,
                                lambda: part2(0), lambda: part2(1)]

                    def emit_qproj(qt):
                        qps = sc2_ps.tile([128, 2, QT], f32, tag="sc2",
                                          name="qps")
                        for b in range(B):
                            isb = make_isb(qt, x_stats[b])
                            proj_matmuls(qps[:, b, :], qt, x_raws[b][qt],
                                         0, wq_sb, x_stats[b])
                            post_scale(qT_sb, b, qt, qps[:, b, :], isb)

                    emit_qproj(0)
                    tail_parts = []      # pending tail pieces of prev qt
                    for qt in range(n_qt):
                        q_sl = slice(qt * QT, (qt + 1) * QT)
                        av = None
                        pend = []        # exm tiles awaiting AV matmuls

                        def flush_av(kt, exms):
                            for b in range(B):
                                for h in range(H_PER_CORE):
                                    nc.tensor.matmul(
                                        av[b][h],
                                        vaug_sb[:, vaug_idx(b, kt, h), 0:65],
                                        exms[b][:, h, :],
                                        start=(kt == 0),
                                        stop=(kt == n_kt - 1))

                        for kt in range(n_kt):
                            k_sl = slice(kt * KT, (kt + 1) * KT)
                            eal = ealp.tile([128, H_PER_CORE, QT], bf16,
                                            tag="eal")
                            nc.sync.dma_start(
                                out=eal, in_=ealT[:, k_sl, q_sl].rearrange(
                                    "h p n -> p h n"))
                            exms = []
                            for b in range(B):
                                sc2 = sc2_ps.tile([128, H_PER_CORE, QT], f32,
                                                  tag="sc2", name="sc2")
                                for h in range(H_PER_CORE):
                                    c_sl = slice(h * 64, (h + 1) * 64)
                                    nc.tensor.matmul(
                                        sc2[:, h, :], kT_sb[c_sl, b, k_sl],
                                        qT_sb[c_sl, b, q_sl],
                                        start=True, stop=True,
                                        tile_position=(h * 64, 0))
                                ex2 = exp_sb.tile([128, H_PER_CORE, QT], bf16,
                                                  tag="ex2")
                                nc.scalar.activation(
                                    out=ex2, in_=sc2,
                                    func=mybir.ActivationFunctionType.Exp,
                                    bias=zeros128[:, 0:1], scale=1.0)
                                exm = exm_sb.tile([128, H_PER_CORE, QT], bf16,
                                                  tag="exm")
                                nc.vector.tensor_mul(exm, ex2, eal)
                                exms.append(exm)
                            pend.append((kt, exms))
                            # interleave previous qt's tail into the first
                            # kt iterations (cross-engine overlap; no PE
                            # head-of-line blocking)
                            if kt < len(tail_parts):
                                tail_parts[kt]()
                            if kt == 1:
                                # av alloc after the last readers (o_sb) of
                                # the previous qt's accumulators are emitted
                                av = [[av_ps.tile([65, QT], f32,
                                                  tag=f"av{b}{h}",
                                                  name=f"av{b}{h}")
                                       for h in range(H_PER_CORE)]
                                      for b in range(B)]
                            if kt == 8 and qt + 1 < n_qt:
                                emit_qproj(qt + 1)
                            if len(pend) > 1 and av is not None:
                                flush_av(*pend.pop(0))
                        while pend:
                            flush_av(*pend.pop(0))
                        tail_parts = make_tail(qt, av)
                    for p in tail_parts:
                        p()
    nc.compile()
    return nc


_NC_CACHE = {}


def _get_program(n_tok=N_TOK, with_pbias=False):
    key = (n_tok, with_pbias)
    if key not in _NC_CACHE:
        _NC_CACHE[key] = build_program(n_tok, with_pbias)
    return _NC_CACHE[key]


def _prep_in_maps(x, context, alibi, Wq, Wk, Wv, Wo, bo, ln_w, ln_b):
    b, n, d = x.shape
    scale = (d // HEADS) ** -0.5

    x = np.asarray(x, dtype=np.float32)
    context = np.asarray(context, dtype=np.float32)
    alibi = np.asarray(alibi, dtype=np.float32)
    Wq, Wk, Wv, Wo = (np.asarray(w, dtype=np.float32) for w in (Wq, Wk, Wv, Wo))
    bo = np.asarray(bo, dtype=np.float32)
    ln_w = np.asarray(ln_w, dtype=np.float32)
    ln_b = np.asarray(ln_b, dtype=np.float32)

    xT = np.ascontiguousarray(x.transpose(0, 2, 1)).astype(BF16)
    cT = np.ascontiguousarray(context.transpose(0, 2, 1)).astype(BF16)

    in_maps = []
    with_pbias = False
    for ci in range(N_CORES):
        h0 = ci * H_PER_CORE
        cs = slice(h0 * DH, (h0 + H_PER_CORE) * DH)  # this core's 128 channels
        ealT = np.exp(np.ascontiguousarray(
            alibi[0, h0:h0 + H_PER_CORE].transpose(0, 2, 1))).astype(BF16)

        wq_s = (Wq[cs] * ln_w[None, :]) * scale          # [128, d]
        wk_s = Wk[cs] * ln_w[None, :]
        wv_s = Wv[cs] * ln_w[None, :]
        wbar = np.stack([
            -wq_s.sum(axis=1), -wk_s.sum(axis=1), -wv_s.sum(axis=1)])
        pb = np.stack([
            (Wq[cs] @ ln_b) * scale, Wk[cs] @ ln_b, Wv[cs] @ ln_b])  # [3,128]
        if np.abs(pb).max() > 0:
            with_pbias = True

        bo_core = bo if ci == 0 else np.zeros_like(bo)

        in_maps.append({
            "xT": xT,
            "cT": cT,
            "ealT": ealT,
            "wqT": np.ascontiguousarray(wq_s.T).astype(BF16),
            "wkT": np.ascontiguousarray(wk_s.T).astype(BF16),
            "wvT": np.ascontiguousarray(wv_s.T).astype(BF16),
            "wbar": wbar.astype(BF16),
            "woT": np.ascontiguousarray(Wo[:, cs].T).astype(BF16),
            "bo_r": np.ascontiguousarray(
                bo_core.reshape(N_DT, 128).T).astype(np.float32),
        })
        if with_pbias:
            in_maps[-1]["pbias"] = pb.astype(BF16)
    if with_pbias:
        for m in in_maps:
            m.setdefault("pbias", np.zeros((3, 128), dtype=BF16))
    return in_maps, with_pbias


def _gather(results, b, n, d):
    acc = np.zeros((d, b * n), dtype=np.float32)
    for r in results:
        acc += r["outT"].astype(np.float32)
    return np.ascontiguousarray(
        acc.reshape(d, b, n).transpose(1, 2, 0)).astype(np.float32)


def kernel(**inputs):
    from concourse.bass_utils import run_bass_kernel_spmd
    x = inputs["x"]
    b, n, d = x.shape
    in_maps, with_pbias = _prep_in_maps(**inputs)
    nc = _get_program(n, with_pbias)
    res = run_bass_kernel_spmd(nc, in_maps, list(range(N_CORES)))
    return _gather(res.results, b, n, d)


def run_profiled(inputs, trace=True):
    from concourse.bass_utils import run_bass_kernel_spmd
    x = inputs["x"]
    b, n, d = x.shape
    in_maps, with_pbias = _prep_in_maps(**inputs)
    nc = _get_program(n, with_pbias)
    res = run_bass_kernel_spmd(nc, in_maps, list(range(N_CORES)), trace=trace)
    return _gather(res.results, b, n, d), res


# revision 23
# speedup vs baseline: 1.0294x; 1.0294x over previous
"""CrossAttention kernel for 8 Trainium2 NeuronCores (Bass/Tile).

Sharding: tensor-parallel over heads. Core i handles heads {2i, 2i+1} for
both batch elements. LayerNorm scale/bias and the q-scale are folded into
the projection weights on the host. Projections run on RAW (un-normalized)
tokens: the per-token mean correction rides the matmul as a K=1 row and
the 1/sigma scaling is applied by one scalar_tensor_tensor per tile.
LN sum-of-squares uses fp8 squares (Act Square) contracted with DoubleRow
matmuls at half cost. exp(alibi) is precomputed on the host (bf16):
softmax weights are exp(scores) * exp(alibi), so the alibi add becomes a
2-byte-fast-path multiply (DVE/Pool) after the exp instead of an f32 PSUM
add. The softmax denominator rides the AV matmul as a ones-column of V.
The q-projections are software-pipelined into the attention phase (one
token tile per query tile). Output partials are written bf16; host gather
sums the 8 partial [dout, tok] projections.
"""

import os
import sys

for _p in ("/opt/trn_rl_repo", "/root/.axon_site/_ro/trn_rl_repo"):
    if os.path.isdir(_p) and _p not in sys.path:
        sys.path.insert(0, _p)

import numpy as np
import ml_dtypes

import concourse.bass as bass
import concourse.tile as tile
from concourse import bacc, mybir
from concourse.masks import make_identity

BF16 = ml_dtypes.bfloat16

HEADS = 16
N_CORES = 8
H_PER_CORE = HEADS // N_CORES  # 2
DH = 64
LN_EPS = 1e-5

B = 2
N_TOK = 2048
D = 1024

QT = 512            # query tile (free dim of scores matmuls)
KT = 128            # key tile (partition dim of scoresT)
TT = 512            # token tile for LN/projection phase
N_DT = D // 128     # 8 contraction tiles of 128 over d

POOL_MUL_MOD = 8    # kt % POOL_MUL_MOD < POOL_MUL_CNT -> eal-mult on Pool
POOL_MUL_CNT = 0


def build_program(n_tok=N_TOK, with_pbias=False):
    """Build the single-core SPMD Bass program. Returns nc."""
    nc = bacc.Bacc("TRN2")
    f32 = mybir.dt.float32
    f32r = mybir.dt.float32r
    bf16 = mybir.dt.bfloat16
    fp8 = mybir.dt.float8e4

    n_tt = n_tok // TT          # token tiles per batch
    n_qt = n_tok // QT          # query tiles per batch
    n_kt = n_tok // KT          # key tiles per batch
    assert n_tt == n_qt  # q-proj pipelined per query tile

    # ---- DRAM parameters (per-core shards, host-prepped) ----
    xT = nc.declare_dram_parameter("xT", [B, D, n_tok], bf16, isOutput=False)
    cT = nc.declare_dram_parameter("cT", [B, D, n_tok], bf16, isOutput=False)
    # exp(alibi), transposed: [h, key, q]
    ealT = nc.declare_dram_parameter(
        "ealT", [H_PER_CORE, n_tok, n_tok], bf16, isOutput=False)
    wqT = nc.declare_dram_parameter("wqT", [D, 128], bf16, isOutput=False)
    wkT = nc.declare_dram_parameter("wkT", [D, 128], bf16, isOutput=False)
    wvT = nc.declare_dram_parameter("wvT", [D, 128], bf16, isOutput=False)
    # rows: -wbar_q, -wbar_k, -wbar_v   (lhsT for the K=1 mu-correction row)
    wbar = nc.declare_dram_parameter("wbar", [3, 128], bf16, isOutput=False)
    woT = nc.declare_dram_parameter("woT", [128, D], bf16, isOutput=False)
    if with_pbias:
        # columns: q/k/v projection bias (ln_b folded through W), bf16 lhsT
        pbias = nc.declare_dram_parameter("pbias", [3, 128], bf16,
                                          isOutput=False)
    bo_r = nc.declare_dram_parameter("bo_r", [128, N_DT], f32, isOutput=False)

    outT = nc.declare_dram_parameter(
        "outT", [D, B * n_tok], bf16, isOutput=True)

    xT_r = xT.rearrange("b (dt p) n -> b p dt n", p=128)
    cT_r = cT.rearrange("b (dt p) n -> b p dt n", p=128)
    woT_r = woT.rearrange("c (dt n) -> c dt n", n=128)

    with tile.TileContext(nc) as tc:
        with tc.tile_pool(name="const", bufs=1) as const_pool:
            ident = const_pool.tile([128, 128], bf16)
            make_identity(nc, ident)
            zeros128 = const_pool.tile([128, 1], f32)
            nc.vector.memset(zeros128, 0.0)
            eps4 = const_pool.tile([4, 1], f32)
            nc.vector.memset(eps4, LN_EPS)
            # stats lhsT: onehot[:, u, j] is all-ones iff j == u
            onehot = const_pool.tile([128, n_tt, 4], bf16)
            nc.vector.memset(onehot, 0.0)
            # fp8 onehot pairs for the DoubleRow sxx matmuls
            # (DoubleRow ldweights requires stationary free dim >= 16)
            onehot8 = const_pool.tile([128, n_tt, 2, 16], fp8)
            nc.vector.memset(onehot8, 0.0)
            for u in range(n_tt):
                nc.vector.memset(onehot[:, u, u:u + 1], 1.0)
                nc.vector.memset(onehot8[:, u, :, u:u + 1], 1.0)

            wq_sb = const_pool.tile([128, N_DT, 128], bf16)
            wk_sb = const_pool.tile([128, N_DT, 128], bf16)
            wv_sb = const_pool.tile([128, N_DT, 128], bf16)
            nc.sync.dma_start(out=wq_sb, in_=wqT.rearrange("(dt p) c -> p dt c", p=128))
            nc.sync.dma_start(out=wk_sb, in_=wkT.rearrange("(dt p) c -> p dt c", p=128))
            nc.sync.dma_start(out=wv_sb, in_=wvT.rearrange("(dt p) c -> p dt c", p=128))
            wbar_sb = const_pool.tile([1, 3, 128], bf16)
            nc.sync.dma_start(out=wbar_sb, in_=wbar[None, :, :])
            wo_sb = const_pool.tile([128, N_DT, 128], bf16)
            nc.sync.dma_start(out=wo_sb, in_=woT_r)
            if with_pbias:
                pbias_sb = const_pool.tile([1, 3, 128], bf16)
                nc.sync.dma_start(out=pbias_sb, in_=pbias[None, :, :])
            bo_sb = const_pool.tile([128, N_DT], f32)
            nc.sync.dma_start(out=bo_sb, in_=bo_r[:, :])

            # persistent activations: [c(128), b, tok]
            qT_sb = const_pool.tile([128, B, n_tok], f32r)
            kT_sb = const_pool.tile([128, B, n_tok], f32r)
            # v natural (+ones col): [key(128), b*n_kt*h, 66]
            vaug_sb = const_pool.tile([128, B * n_kt * H_PER_CORE, 66], bf16)
            nc.vector.memset(vaug_sb[:, :, 64:65], 1.0)

            def vaug_idx(b, kt, h):
                return (b * n_kt + kt) * H_PER_CORE + h

            # SBUF pools that live for the whole program (x raws are read
            # during the attention phase by the pipelined q-projections)
            with tc.tile_pool(name="raw_p", bufs=2 * n_tt + 1) as raw_p, \
                 tc.tile_pool(name="sq8_p", bufs=2) as sq8_p, \
                 tc.tile_pool(name="pha", bufs=2) as pha, \
                 tc.tile_pool(name="stat_tmp", bufs=1) as stat_tmp, \
                 tc.tile_pool(name="stat_sb", bufs=2) as stat_sb:

                def ln_stats(raws):
                    """DMA'd raw tiles -> (m_row, inv_row[, s_row]) rows."""
                    sx = stat_ps.tile([4, TT], f32, tag="sx")
                    sxx16 = stat_ps.tile([16, TT], f32, tag="sxx")
                    sxx = sxx16[0:4, :]
                    for u in range(n_tt):
                        sq8 = sq8_p.tile([128, N_DT, TT], fp8, tag="sq8")
                        nc.scalar.activation(
                            out=sq8, in_=raws[u],
                            func=mybir.ActivationFunctionType.Square,
                            bias=zeros128[:, 0:1], scale=1.0)
                        for dt in range(N_DT):
                            nc.tensor.matmul(
                                sx, onehot[:, u, :], raws[u][:, dt, :],
                                start=(u == 0 and dt == 0),
                                stop=(u == n_tt - 1 and dt == N_DT - 1))
                        for dp in range(N_DT // 2):
                            nc.tensor.matmul(
                                sxx16, onehot8[:, u, :, :],
                                sq8[:, dp * 2:dp * 2 + 2, :],
                                start=(u == 0 and dp == 0),
                                stop=(u == n_tt - 1 and dp == N_DT // 2 - 1),
                                perf_mode=mybir.MatmulPerfMode.DoubleRow)
                    # batched LN math on [n_tt, TT] rows
                    e = stat_tmp.tile([4, TT], f32, tag="e")
                    nc.vector.tensor_scalar_mul(e, sx, 1.0 / D)
                    ee = stat_tmp.tile([4, TT], f32, tag="ee")
                    nc.vector.tensor_mul(ee, e, e)
                    var = stat_tmp.tile([4, TT], f32, tag="var")
                    # var*D = Sxx - D*ee
                    nc.vector.scalar_tensor_tensor(
                        out=var, in0=ee, scalar=float(-D), in1=sxx,
                        op0=mybir.AluOpType.mult, op1=mybir.AluOpType.add)
                    sig0 = stat_tmp.tile([4, TT], f32, tag="sig0")
                    nc.scalar.activation(
                        out=sig0, in_=var,
                        func=mybir.ActivationFunctionType.Sqrt,
                        bias=eps4[:, 0:1], scale=1.0 / D)
                    invs = stat_tmp.tile([4, TT], f32, tag="invs")
                    nc.vector.reciprocal(invs, sig0)
                    invs_bf = stat_tmp.tile([4, TT], bf16, tag="invs_bf")
                    nc.vector.tensor_copy(invs_bf, invs)
                    m_bf = stat_tmp.tile([4, TT], bf16, tag="m_bf")
                    nc.vector.tensor_copy(m_bf, e)
                    # restage rows at partition 0 (matmul rhs and
                    # partition_broadcast both need base partition 0)
                    m_row = stat_sb.tile([1, n_tt, TT], bf16, tag="m_row")
                    inv_row = stat_sb.tile([1, n_tt, TT], bf16, tag="inv_row")
                    for u in range(n_tt):
                        nc.sync.dma_start(
                            out=m_row[:, u, :], in_=m_bf[u:u + 1, :])
                        nc.sync.dma_start(
                            out=inv_row[:, u, :], in_=invs_bf[u:u + 1, :])
                    if with_pbias:
                        # sigma row: pbias must not be scaled by 1/sigma
                        sig_bf = stat_tmp.tile([4, TT], bf16, tag="sig_bf")
                        nc.vector.tensor_copy(sig_bf, sig0)
                        s_row = stat_sb.tile([1, n_tt, TT], bf16, tag="s_row")
                        for u in range(n_tt):
                            nc.sync.dma_start(
                                out=s_row[:, u, :], in_=sig_bf[u:u + 1, :])
                    else:
                        s_row = None
                    return m_row, inv_row, s_row

                def proj_matmuls(ps, u, raw, wi, w_sb, stats):
                    m_row, _, s_row = stats
                    for dt in range(N_DT):
                        nc.tensor.matmul(
                            ps, w_sb[:, dt, :], raw[:, dt, :],
                            start=(dt == 0), stop=False)
                    nc.tensor.matmul(
                        ps, wbar_sb[:, wi, :], m_row[:, u, :],
                        start=False, stop=not with_pbias)
                    if with_pbias:
                        nc.tensor.matmul(
                            ps, pbias_sb[:, wi, :], s_row[:, u, :],
                            start=False, stop=True)

                def make_isb(u, stats):
                    isb = pha.tile([128, TT], bf16, tag="isb")
                    nc.gpsimd.partition_broadcast(isb, stats[1][:, u, :])
                    return isb

                def post_scale(dst, b, u, ps, isb):
                    nc.vector.scalar_tensor_tensor(
                        out=dst[:, b, u * TT:(u + 1) * TT],
                        in0=ps, scalar=1.0, in1=isb,
                        op0=mybir.AluOpType.mult, op1=mybir.AluOpType.mult)

                # ====== Phase A: context stats + k/v projections =========
                with tc.tile_pool(name="vtp", bufs=1) as vtp, \
                     tc.tile_pool(name="pha_ps", bufs=2, space="PSUM") as pha_ps, \
                     tc.tile_pool(name="stat_ps", bufs=2, space="PSUM") as stat_ps, \
                     tc.tile_pool(name="vt_ps", bufs=2, space="PSUM") as vt_ps:
                    vT_sb = vtp.tile([128, B, n_tok], bf16)
                    for b in range(B):
                        raws = []
                        for u in range(n_tt):
                            raw = raw_p.tile([128, N_DT, TT], bf16, tag="raw")
                            raws.append(raw)
                            nc.sync.dma_start(
                                out=raw,
                                in_=cT_r[b, :, :, u * TT:(u + 1) * TT])
                        stats = ln_stats(raws)
                        for u in range(n_tt):
                            isb = make_isb(u, stats)
                            for wi, w_sb, dst in ((1, wk_sb, kT_sb),
                                                  (2, wv_sb, vT_sb)):
                                ps = pha_ps.tile([128, TT], f32, tag="proj")
                                proj_matmuls(ps, u, raws[u], wi, w_sb, stats)
                                post_scale(dst, b, u, ps, isb)
                        # v natural (transpose vT) once per ctx batch
                        for kt in range(n_kt):
                            vt = vt_ps.tile([128, 128], bf16, tag="vt")
                            nc.tensor.transpose(
                                vt, vT_sb[:, b, kt * KT:(kt + 1) * KT], ident)
                            # both heads in one copy (adjacent vaug rows)
                            nc.vector.tensor_copy(
                                vaug_sb[:, vaug_idx(b, kt, 0):
                                        vaug_idx(b, kt, 0) + 2, 0:64],
                                vt.rearrange("p (h c) -> p h c", h=2))

                    # x raws + stats (q projections are pipelined into
                    # the attention phase below)
                    x_raws = []
                    for b in range(B):
                        raws = []
                        for u in range(n_tt):
                            raw = raw_p.tile([128, N_DT, TT], bf16, tag="raw")
                            raws.append(raw)
                            nc.sync.dma_start(
                                out=raw,
                                in_=xT_r[b, :, :, u * TT:(u + 1) * TT])
                        x_raws.append(raws)
                    x_stats = [ln_stats(x_raws[b]) for b in range(B)]

                # ====== Phase B: attention + output projection ===========
                with tc.tile_pool(name="ealp", bufs=2) as ealp, \
                     tc.tile_pool(name="exp_sb", bufs=3) as exp_sb, \
                     tc.tile_pool(name="exm_sb", bufs=5) as exm_sb, \
                     tc.tile_pool(name="phbd", bufs=1) as phbd, \
                     tc.tile_pool(name="phfo", bufs=2) as phfo, \
                     tc.tile_pool(name="sc2_ps", bufs=2, space="PSUM") as sc2_ps, \
                     tc.tile_pool(name="av_ps", bufs=1, space="PSUM") as av_ps:
                    def make_tail(qt, av):
                        q0 = qt * QT

                        def part0():
                            for b in range(B):
                                den = phbd.tile([1, 2, QT], f32, tag="den",
                                                name=f"den{b}")
                                for h in range(H_PER_CORE):
                                    nc.vector.tensor_copy(
                                        den[:, h, :], av[b][h][64:65, :])
                                rden = phbd.tile([1, 2, QT], f32, tag="rden",
                                                 name=f"rden{b}")
                                nc.vector.reciprocal_approx_fast(rden, den)
                                for h in range(H_PER_CORE):
                                    rb = phbd.tile([64, QT], f32,
                                                   tag=f"rb{b}{h}")
                                    nc.gpsimd.partition_broadcast(
                                        rb, rden[:, h, :])
                                    st[f"rb{b}{h}"] = rb

                        def part1():
                            for b in range(B):
                                o_sb = phbd.tile([128, QT], bf16,
                                                 tag=f"o_sb{b}")
                                for h in range(H_PER_CORE):
                                    nc.vector.tensor_mul(
                                        o_sb[h * 64:(h + 1) * 64, :],
                                        av[b][h][0:64, :], st[f"rb{b}{h}"])
                                st[f"o_sb{b}"] = o_sb

                        def part2(b, dp):
                            o_sb = st[f"o_sb{b}"]
                            outT_r = outT.rearrange("(dt p) n -> p dt n",
                                                    p=128)
                            fp2 = sc2_ps.tile([128, 2, QT], f32,
                                              tag="sc2", name="fp2")
                            fo = phfo.tile([128, 2, QT], bf16, tag="fo")
                            for j in range(2):
                                dt = dp * 2 + j
                                nc.tensor.matmul(
                                    fp2[:, j, :], wo_sb[:, dt, :], o_sb,
                                    start=True, stop=True)
                                if j == 0:
                                    nc.scalar.activation(
                                        out=fo[:, j, :], in_=fp2[:, j, :],
                                        func=mybir.ActivationFunctionType
                                        .Identity,
                                        bias=bo_sb[:, dt:dt + 1],
                                        scale=1.0)
                                else:
                                    nc.vector.tensor_scalar_add(
                                        fo[:, j, :], fp2[:, j, :],
                                        bo_sb[:, dt:dt + 1])
                            nc.sync.dma_start(
                                out=outT_r[
                                    :, dp * 2:dp * 2 + 2,
                                    b * n_tok + q0:b * n_tok + q0 + QT],
                                in_=fo)

                        st = {}
                        return [part0, part1] + [
                            (lambda b=b, dp=dp: part2(b, dp))
                            for b in range(B) for dp in range(N_DT // 2)]

                    def emit_qproj(qt):
                        qps = sc2_ps.tile([128, 2, QT], f32, tag="sc2",
                                          name="qps")
                        for b in range(B):
                            isb = make_isb(qt, x_stats[b])
                            proj_matmuls(qps[:, b, :], qt, x_raws[b][qt],
                                         0, wq_sb, x_stats[b])
                            post_scale(qT_sb, b, qt, qps[:, b, :], isb)

                    emit_qproj(0)
                    tail_parts = []      # pending tail pieces of prev qt
                    for qt in range(n_qt):
                        q_sl = slice(qt * QT, (qt + 1) * QT)
                        av = None
                        pend = []        # exm tiles awaiting AV matmuls

                        def flush_av(kt, exms):
                            for b in range(B):
                                for h in range(H_PER_CORE):
                                    nc.tensor.matmul(
                                        av[b][h],
                                        vaug_sb[:, vaug_idx(b, kt, h), 0:65],
                                        exms[b][:, h, :],
                                        start=(kt == 0),
                                        stop=(kt == n_kt - 1))

                        for kt in range(n_kt):
                            k_sl = slice(kt * KT, (kt + 1) * KT)
                            eal = ealp.tile([128, H_PER_CORE, QT], bf16,
                                            tag="eal")
                            nc.sync.dma_start(
                                out=eal, in_=ealT[:, k_sl, q_sl].rearrange(
                                    "h p n -> p h n"))
                            exms = []
                            for b in range(B):
                                sc2 = sc2_ps.tile([128, H_PER_CORE, QT], f32,
                                                  tag="sc2", name="sc2")
                                for h in range(H_PER_CORE):
                                    c_sl = slice(h * 64, (h + 1) * 64)
                                    nc.tensor.matmul(
                                        sc2[:, h, :], kT_sb[c_sl, b, k_sl],
                                        qT_sb[c_sl, b, q_sl],
                                        start=True, stop=True,
                                        tile_position=(h * 64, 0))
                                ex2 = exp_sb.tile([128, H_PER_CORE, QT], bf16,
                                                  tag="ex2")
                                nc.scalar.activation(
                                    out=ex2, in_=sc2,
                                    func=mybir.ActivationFunctionType.Exp,
                                    bias=zeros128[:, 0:1], scale=1.0)
                                exm = exm_sb.tile([128, H_PER_CORE, QT], bf16,
                                                  tag="exm")
                                nc.vector.tensor_mul(exm, ex2, eal)
                                exms.append(exm)
                            pend.append((kt, exms))
                            # interleave previous qt's tail into the first
                            # kt iterations (cross-engine overlap; no PE
                            # head-of-line blocking)
                            if kt < len(tail_parts):
                                tail_parts[kt]()
                            if kt == 1:
                                # av alloc after the last readers (o_sb) of
                                # the previous qt's accumulators are emitted
                                av = [[av_ps.tile([65, QT], f32,
                                                  tag=f"av{b}{h}",
                                                  name=f"av{b}{h}")
                                       for h in range(H_PER_CORE)]
                                      for b in range(B)]
                            if kt == 11 and qt + 1 < n_qt:
                                emit_qproj(qt + 1)
                            if len(pend) > 1 and av is not None:
                                flush_av(*pend.pop(0))
                        while pend:
                            flush_av(*pend.pop(0))
                        tail_parts = make_tail(qt, av)
                    for p in tail_parts:
                        p()
    nc.compile()
    return nc


_NC_CACHE = {}


def _get_program(n_tok=N_TOK, with_pbias=False):
    key = (n_tok, with_pbias)
    if key not in _NC_CACHE:
        _NC_CACHE[key] = build_program(n_tok, with_pbias)
    return _NC_CACHE[key]


def _prep_in_maps(x, context, alibi, Wq, Wk, Wv, Wo, bo, ln_w, ln_b):
    b, n, d = x.shape
    scale = (d // HEADS) ** -0.5

    x = np.asarray(x, dtype=np.float32)
    context = np.asarray(context, dtype=np.float32)
    alibi = np.asarray(alibi, dtype=np.float32)
    Wq, Wk, Wv, Wo = (np.asarray(w, dtype=np.float32) for w in (Wq, Wk, Wv, Wo))
    bo = np.asarray(bo, dtype=np.float32)
    ln_w = np.asarray(ln_w, dtype=np.float32)
    ln_b = np.asarray(ln_b, dtype=np.float32)

    xT = np.ascontiguousarray(x.transpose(0, 2, 1)).astype(BF16)
    cT = np.ascontiguousarray(context.transpose(0, 2, 1)).astype(BF16)

    in_maps = []
    with_pbias = False
    for ci in range(N_CORES):
        h0 = ci * H_PER_CORE
        cs = slice(h0 * DH, (h0 + H_PER_CORE) * DH)  # this core's 128 channels
        ealT = np.exp(np.ascontiguousarray(
            alibi[0, h0:h0 + H_PER_CORE].transpose(0, 2, 1))).astype(BF16)

        wq_s = (Wq[cs] * ln_w[None, :]) * scale          # [128, d]
        wk_s = Wk[cs] * ln_w[None, :]
        wv_s = Wv[cs] * ln_w[None, :]
        wbar = np.stack([
            -wq_s.sum(axis=1), -wk_s.sum(axis=1), -wv_s.sum(axis=1)])
        pb = np.stack([
            (Wq[cs] @ ln_b) * scale, Wk[cs] @ ln_b, Wv[cs] @ ln_b])  # [3,128]
        if np.abs(pb).max() > 0:
            with_pbias = True

        bo_core = bo if ci == 0 else np.zeros_like(bo)

        in_maps.append({
            "xT": xT,
            "cT": cT,
            "ealT": ealT,
            "wqT": np.ascontiguousarray(wq_s.T).astype(BF16),
            "wkT": np.ascontiguousarray(wk_s.T).astype(BF16),
            "wvT": np.ascontiguousarray(wv_s.T).astype(BF16),
            "wbar": wbar.astype(BF16),
            "woT": np.ascontiguousarray(Wo[:, cs].T).astype(BF16),
            "bo_r": np.ascontiguousarray(
                bo_core.reshape(N_DT, 128).T).astype(np.float32),
        })
        if with_pbias:
            in_maps[-1]["pbias"] = pb.astype(BF16)
    if with_pbias:
        for m in in_maps:
            m.setdefault("pbias", np.zeros((3, 128), dtype=BF16))
    return in_maps, with_pbias


def _gather(results, b, n, d):
    acc = np.zeros((d, b * n), dtype=np.float32)
    for r in results:
        acc += r["outT"].astype(np.float32)
    return np.ascontiguousarray(
        acc.reshape(d, b, n).transpose(1, 2, 0)).astype(np.float32)


def kernel(**inputs):
    from concourse.bass_utils import run_bass_kernel_spmd
    x = inputs["x"]
    b, n, d = x.shape
    in_maps, with_pbias = _prep_in_maps(**inputs)
    nc = _get_program(n, with_pbias)
    res = run_bass_kernel_spmd(nc, in_maps, list(range(N_CORES)))
    return _gather(res.results, b, n, d)


def run_profiled(inputs, trace=True):
    from concourse.bass_utils import run_bass_kernel_spmd
    x = inputs["x"]
    b, n, d = x.shape
    in_maps, with_pbias = _prep_in_maps(**inputs)
    nc = _get_program(n, with_pbias)
    res = run_bass_kernel_spmd(nc, in_maps, list(range(N_CORES)), trace=trace)
    return _gather(res.results, b, n, d), res


# revision 24
# speedup vs baseline: 1.0730x; 1.0424x over previous
"""CrossAttention kernel for 8 Trainium2 NeuronCores (Bass/Tile).

Sharding: tensor-parallel over heads. Core i handles heads {2i, 2i+1} for
both batch elements. LayerNorm scale/bias and the q-scale are folded into
the projection weights on the host. Projections run on RAW (un-normalized)
tokens: the per-token mean correction rides the matmul as a K=1 row and
the 1/sigma scaling is applied by one scalar_tensor_tensor per tile.
LN sum-of-squares uses fp8 squares (Act Square) contracted with DoubleRow
matmuls at half cost. exp(alibi) is precomputed on the host (bf16):
softmax weights are exp(scores) * exp(alibi), so the alibi add becomes a
2-byte-fast-path multiply (DVE/Pool) after the exp instead of an f32 PSUM
add. The softmax denominator rides the AV matmul as a ones-column of V.
The q-projections are software-pipelined into the attention phase (one
token tile per query tile). Output partials are written bf16; host gather
sums the 8 partial [dout, tok] projections.
"""

import os
import sys

for _p in ("/opt/trn_rl_repo", "/root/.axon_site/_ro/trn_rl_repo"):
    if os.path.isdir(_p) and _p not in sys.path:
        sys.path.insert(0, _p)

import numpy as np
import ml_dtypes

import concourse.bass as bass
import concourse.tile as tile
from concourse import bacc, mybir
from concourse.masks import make_identity

BF16 = ml_dtypes.bfloat16

HEADS = 16
N_CORES = 8
H_PER_CORE = HEADS // N_CORES  # 2
DH = 64
LN_EPS = 1e-5

B = 2
N_TOK = 2048
D = 1024

QT = 512            # query tile (free dim of scores matmuls)
KT = 128            # key tile (partition dim of scoresT)
TT = 512            # token tile for LN/projection phase
N_DT = D // 128     # 8 contraction tiles of 128 over d

POOL_MUL_MOD = 8    # kt % POOL_MUL_MOD < POOL_MUL_CNT -> eal-mult on Pool
POOL_MUL_CNT = 0


def build_program(n_tok=N_TOK, with_pbias=False):
    """Build the single-core SPMD Bass program. Returns nc."""
    nc = bacc.Bacc("TRN2")
    f32 = mybir.dt.float32
    f32r = mybir.dt.float32r
    bf16 = mybir.dt.bfloat16
    fp8 = mybir.dt.float8e4

    n_tt = n_tok // TT          # token tiles per batch
    n_qt = n_tok // QT          # query tiles per batch
    n_kt = n_tok // KT          # key tiles per batch
    assert n_tt == n_qt  # q-proj pipelined per query tile

    # ---- DRAM parameters (per-core shards, host-prepped) ----
    xT = nc.declare_dram_parameter("xT", [B, D, n_tok], bf16, isOutput=False)
    cT = nc.declare_dram_parameter("cT", [B, D, n_tok], bf16, isOutput=False)
    # exp(alibi), transposed: [h, key, q]
    ealT = nc.declare_dram_parameter(
        "ealT", [H_PER_CORE, n_tok, n_tok], bf16, isOutput=False)
    wqT = nc.declare_dram_parameter("wqT", [D, 128], bf16, isOutput=False)
    wkT = nc.declare_dram_parameter("wkT", [D, 128], bf16, isOutput=False)
    wvT = nc.declare_dram_parameter("wvT", [D, 128], bf16, isOutput=False)
    # rows: -wbar_q, -wbar_k, -wbar_v   (lhsT for the K=1 mu-correction row)
    wbar = nc.declare_dram_parameter("wbar", [3, 128], bf16, isOutput=False)
    woT = nc.declare_dram_parameter("woT", [128, D], bf16, isOutput=False)
    if with_pbias:
        # columns: q/k/v projection bias (ln_b folded through W), bf16 lhsT
        pbias = nc.declare_dram_parameter("pbias", [3, 128], bf16,
                                          isOutput=False)
    bo_r = nc.declare_dram_parameter("bo_r", [128, N_DT], f32, isOutput=False)

    outT = nc.declare_dram_parameter(
        "outT", [D, B * n_tok], bf16, isOutput=True)

    xT_r = xT.rearrange("b (dt p) n -> b p dt n", p=128)
    cT_r = cT.rearrange("b (dt p) n -> b p dt n", p=128)
    woT_r = woT.rearrange("c (dt n) -> c dt n", n=128)

    with tile.TileContext(nc) as tc:
        with tc.tile_pool(name="const", bufs=1) as const_pool:
            ident = const_pool.tile([128, 128], bf16)
            make_identity(nc, ident)
            zeros128 = const_pool.tile([128, 1], f32)
            nc.vector.memset(zeros128, 0.0)
            eps4 = const_pool.tile([4, 1], f32)
            nc.vector.memset(eps4, LN_EPS)
            # stats lhsT: onehot[:, u, j] is all-ones iff j == u
            onehot = const_pool.tile([128, n_tt, 4], bf16)
            nc.vector.memset(onehot, 0.0)
            # fp8 onehot pairs for the DoubleRow sxx matmuls
            # (DoubleRow ldweights requires stationary free dim >= 16)
            onehot8 = const_pool.tile([128, n_tt, 2, 16], fp8)
            nc.vector.memset(onehot8, 0.0)
            for u in range(n_tt):
                nc.vector.memset(onehot[:, u, u:u + 1], 1.0)
                nc.vector.memset(onehot8[:, u, :, u:u + 1], 1.0)

            wq_sb = const_pool.tile([128, N_DT, 128], bf16)
            wk_sb = const_pool.tile([128, N_DT, 128], bf16)
            wv_sb = const_pool.tile([128, N_DT, 128], bf16)
            nc.sync.dma_start(out=wq_sb, in_=wqT.rearrange("(dt p) c -> p dt c", p=128))
            nc.sync.dma_start(out=wk_sb, in_=wkT.rearrange("(dt p) c -> p dt c", p=128))
            nc.sync.dma_start(out=wv_sb, in_=wvT.rearrange("(dt p) c -> p dt c", p=128))
            wbar_sb = const_pool.tile([1, 3, 128], bf16)
            nc.sync.dma_start(out=wbar_sb, in_=wbar[None, :, :])
            wo_sb = const_pool.tile([128, N_DT, 128], bf16)
            nc.sync.dma_start(out=wo_sb, in_=woT_r)
            if with_pbias:
                pbias_sb = const_pool.tile([1, 3, 128], bf16)
                nc.sync.dma_start(out=pbias_sb, in_=pbias[None, :, :])
            bo_sb = const_pool.tile([128, N_DT], f32)
            nc.sync.dma_start(out=bo_sb, in_=bo_r[:, :])

            # persistent activations: [c(128), b, tok]
            qT_sb = const_pool.tile([128, B, n_tok], f32r)
            kT_sb = const_pool.tile([128, B, n_tok], f32r)
            # v natural (+ones col): [key(128), b*n_kt*h, 66]
            vaug_sb = const_pool.tile([128, B * n_kt * H_PER_CORE, 66], bf16)
            nc.vector.memset(vaug_sb[:, :, 64:65], 1.0)

            def vaug_idx(b, kt, h):
                return (b * n_kt + kt) * H_PER_CORE + h

            # SBUF pools that live for the whole program (x raws are read
            # during the attention phase by the pipelined q-projections)
            with tc.tile_pool(name="raw_p", bufs=2 * n_tt + 1) as raw_p, \
                 tc.tile_pool(name="sq8_p", bufs=2) as sq8_p, \
                 tc.tile_pool(name="pha", bufs=2) as pha, \
                 tc.tile_pool(name="stat_tmp", bufs=1) as stat_tmp, \
                 tc.tile_pool(name="stat_sb", bufs=2) as stat_sb:

                def ln_stats(raws):
                    """DMA'd raw tiles -> (m_row, inv_row[, s_row]) rows."""
                    sx = stat_ps.tile([4, TT], f32, tag="sx")
                    sxx16 = stat_ps.tile([16, TT], f32, tag="sxx")
                    sxx = sxx16[0:4, :]
                    for u in range(n_tt):
                        sq8 = sq8_p.tile([128, N_DT, TT], fp8, tag="sq8")
                        nc.scalar.activation(
                            out=sq8, in_=raws[u],
                            func=mybir.ActivationFunctionType.Square,
                            bias=zeros128[:, 0:1], scale=1.0)
                        for dt in range(N_DT):
                            nc.tensor.matmul(
                                sx, onehot[:, u, :], raws[u][:, dt, :],
                                start=(u == 0 and dt == 0),
                                stop=(u == n_tt - 1 and dt == N_DT - 1))
                        for dp in range(N_DT // 2):
                            nc.tensor.matmul(
                                sxx16, onehot8[:, u, :, :],
                                sq8[:, dp * 2:dp * 2 + 2, :],
                                start=(u == 0 and dp == 0),
                                stop=(u == n_tt - 1 and dp == N_DT // 2 - 1),
                                perf_mode=mybir.MatmulPerfMode.DoubleRow)
                    # batched LN math on [n_tt, TT] rows
                    e = stat_tmp.tile([4, TT], f32, tag="e")
                    nc.vector.tensor_scalar_mul(e, sx, 1.0 / D)
                    ee = stat_tmp.tile([4, TT], f32, tag="ee")
                    nc.vector.tensor_mul(ee, e, e)
                    var = stat_tmp.tile([4, TT], f32, tag="var")
                    # var*D = Sxx - D*ee
                    nc.vector.scalar_tensor_tensor(
                        out=var, in0=ee, scalar=float(-D), in1=sxx,
                        op0=mybir.AluOpType.mult, op1=mybir.AluOpType.add)
                    sig0 = stat_tmp.tile([4, TT], f32, tag="sig0")
                    nc.scalar.activation(
                        out=sig0, in_=var,
                        func=mybir.ActivationFunctionType.Sqrt,
                        bias=eps4[:, 0:1], scale=1.0 / D)
                    invs = stat_tmp.tile([4, TT], f32, tag="invs")
                    nc.vector.reciprocal(invs, sig0)
                    invs_bf = stat_tmp.tile([4, TT], bf16, tag="invs_bf")
                    nc.vector.tensor_copy(invs_bf, invs)
                    m_bf = stat_tmp.tile([4, TT], bf16, tag="m_bf")
                    nc.vector.tensor_copy(m_bf, e)
                    # restage rows at partition 0 (matmul rhs and
                    # partition_broadcast both need base partition 0)
                    m_row = stat_sb.tile([1, n_tt, TT], bf16, tag="m_row")
                    inv_row = stat_sb.tile([1, n_tt, TT], bf16, tag="inv_row")
                    for u in range(n_tt):
                        nc.sync.dma_start(
                            out=m_row[:, u, :], in_=m_bf[u:u + 1, :])
                        nc.sync.dma_start(
                            out=inv_row[:, u, :], in_=invs_bf[u:u + 1, :])
                    if with_pbias:
                        # sigma row: pbias must not be scaled by 1/sigma
                        sig_bf = stat_tmp.tile([4, TT], bf16, tag="sig_bf")
                        nc.vector.tensor_copy(sig_bf, sig0)
                        s_row = stat_sb.tile([1, n_tt, TT], bf16, tag="s_row")
                        for u in range(n_tt):
                            nc.sync.dma_start(
                                out=s_row[:, u, :], in_=sig_bf[u:u + 1, :])
                    else:
                        s_row = None
                    return m_row, inv_row, s_row

                def proj_matmuls(ps, u, raw, wi, w_sb, stats):
                    m_row, _, s_row = stats
                    for dt in range(N_DT):
                        nc.tensor.matmul(
                            ps, w_sb[:, dt, :], raw[:, dt, :],
                            start=(dt == 0), stop=False)
                    nc.tensor.matmul(
                        ps, wbar_sb[:, wi, :], m_row[:, u, :],
                        start=False, stop=not with_pbias)
                    if with_pbias:
                        nc.tensor.matmul(
                            ps, pbias_sb[:, wi, :], s_row[:, u, :],
                            start=False, stop=True)

                def make_isb(u, stats):
                    isb = pha.tile([128, TT], bf16, tag="isb")
                    nc.gpsimd.partition_broadcast(isb, stats[1][:, u, :])
                    return isb

                def post_scale(dst, b, u, ps, isb):
                    nc.vector.scalar_tensor_tensor(
                        out=dst[:, b, u * TT:(u + 1) * TT],
                        in0=ps, scalar=1.0, in1=isb,
                        op0=mybir.AluOpType.mult, op1=mybir.AluOpType.mult)

                # ====== Phase A: context stats + k/v projections =========
                with tc.tile_pool(name="vtp", bufs=1) as vtp, \
                     tc.tile_pool(name="pha_ps", bufs=2, space="PSUM") as pha_ps, \
                     tc.tile_pool(name="stat_ps", bufs=2, space="PSUM") as stat_ps, \
                     tc.tile_pool(name="vt_ps", bufs=2, space="PSUM") as vt_ps:
                    vT_sb = vtp.tile([128, B, n_tok], bf16)
                    for b in range(B):
                        raws = []
                        for u in range(n_tt):
                            raw = raw_p.tile([128, N_DT, TT], bf16, tag="raw")
                            raws.append(raw)
                            nc.sync.dma_start(
                                out=raw,
                                in_=cT_r[b, :, :, u * TT:(u + 1) * TT])
                        stats = ln_stats(raws)
                        for u in range(n_tt):
                            isb = make_isb(u, stats)
                            for wi, w_sb, dst in ((1, wk_sb, kT_sb),
                                                  (2, wv_sb, vT_sb)):
                                ps = pha_ps.tile([128, TT], f32, tag="proj")
                                proj_matmuls(ps, u, raws[u], wi, w_sb, stats)
                                post_scale(dst, b, u, ps, isb)
                        # v natural (transpose vT) once per ctx batch
                        for kt in range(n_kt):
                            vt = vt_ps.tile([128, 128], bf16, tag="vt")
                            nc.tensor.transpose(
                                vt, vT_sb[:, b, kt * KT:(kt + 1) * KT], ident)
                            # both heads in one copy (adjacent vaug rows)
                            nc.vector.tensor_copy(
                                vaug_sb[:, vaug_idx(b, kt, 0):
                                        vaug_idx(b, kt, 0) + 2, 0:64],
                                vt.rearrange("p (h c) -> p h c", h=2))

                    # x raws + stats (q projections are pipelined into
                    # the attention phase below)
                    x_raws = []
                    for b in range(B):
                        raws = []
                        for u in range(n_tt):
                            raw = raw_p.tile([128, N_DT, TT], bf16, tag="raw")
                            raws.append(raw)
                            nc.sync.dma_start(
                                out=raw,
                                in_=xT_r[b, :, :, u * TT:(u + 1) * TT])
                        x_raws.append(raws)
                    x_stats = [ln_stats(x_raws[b]) for b in range(B)]

                # ====== Phase B: attention + output projection ===========
                with tc.tile_pool(name="ealp", bufs=2) as ealp, \
                     tc.tile_pool(name="exp_sb", bufs=3) as exp_sb, \
                     tc.tile_pool(name="exm_sb", bufs=5) as exm_sb, \
                     tc.tile_pool(name="phbd", bufs=1) as phbd, \
                     tc.tile_pool(name="phfo", bufs=2) as phfo, \
                     tc.tile_pool(name="sc2_ps", bufs=2, space="PSUM") as sc2_ps, \
                     tc.tile_pool(name="av_ps", bufs=1, space="PSUM") as av_ps:
                    def make_tail(qt, av):
                        q0 = qt * QT

                        def part0():
                            for b in range(B):
                                den = phbd.tile([1, 2, QT], f32, tag="den",
                                                name=f"den{b}")
                                for h in range(H_PER_CORE):
                                    nc.vector.tensor_copy(
                                        den[:, h, :], av[b][h][64:65, :])
                                rden = phbd.tile([1, 2, QT], f32, tag="rden",
                                                 name=f"rden{b}")
                                nc.vector.reciprocal_approx_fast(rden, den)
                                for h in range(H_PER_CORE):
                                    rb = phbd.tile([64, QT], f32,
                                                   tag=f"rb{b}{h}")
                                    nc.gpsimd.partition_broadcast(
                                        rb, rden[:, h, :])
                                    st[f"rb{b}{h}"] = rb

                        def part1():
                            for b in range(B):
                                o_sb = phbd.tile([128, QT], bf16,
                                                 tag=f"o_sb{b}")
                                for h in range(H_PER_CORE):
                                    nc.vector.tensor_mul(
                                        o_sb[h * 64:(h + 1) * 64, :],
                                        av[b][h][0:64, :], st[f"rb{b}{h}"])
                                st[f"o_sb{b}"] = o_sb

                        def part2(b, dp):
                            o_sb = st[f"o_sb{b}"]
                            outT_r = outT.rearrange("(dt p) n -> p dt n",
                                                    p=128)
                            fp2 = sc2_ps.tile([128, 2, QT], f32,
                                              tag="sc2", name="fp2")
                            fo = phfo.tile([128, 2, QT], bf16, tag="fo")
                            for j in range(2):
                                dt = dp * 2 + j
                                nc.tensor.matmul(
                                    fp2[:, j, :], wo_sb[:, dt, :], o_sb,
                                    start=True, stop=True)
                                if j == 0:
                                    nc.scalar.activation(
                                        out=fo[:, j, :], in_=fp2[:, j, :],
                                        func=mybir.ActivationFunctionType
                                        .Identity,
                                        bias=bo_sb[:, dt:dt + 1],
                                        scale=1.0)
                                else:
                                    nc.vector.tensor_scalar_add(
                                        fo[:, j, :], fp2[:, j, :],
                                        bo_sb[:, dt:dt + 1])
                            nc.sync.dma_start(
                                out=outT_r[
                                    :, dp * 2:dp * 2 + 2,
                                    b * n_tok + q0:b * n_tok + q0 + QT],
                                in_=fo)

                        st = {}
                        return [part0, part1] + [
                            (lambda b=b, dp=dp: part2(b, dp))
                            for b in range(B) for dp in range(N_DT // 2)]

                    def emit_qproj(qt):
                        qps = sc2_ps.tile([128, 2, QT], f32, tag="sc2",
                                          name="qps")
                        for b in range(B):
                            isb = make_isb(qt, x_stats[b])
                            proj_matmuls(qps[:, b, :], qt, x_raws[b][qt],
                                         0, wq_sb, x_stats[b])
                            post_scale(qT_sb, b, qt, qps[:, b, :], isb)

                    emit_qproj(0)
                    tail_parts = []      # pending tail pieces of prev qt
                    for qt in range(n_qt):
                        q_sl = slice(qt * QT, (qt + 1) * QT)
                        av = None
                        pend = []        # exm tiles awaiting AV matmuls

                        def flush_av(kt, exms):
                            for b in range(B):
                                for h in range(H_PER_CORE):
                                    nc.tensor.matmul(
                                        av[b][h],
                                        vaug_sb[:, vaug_idx(b, kt, h), 0:65],
                                        exms[b][:, h, :],
                                        start=(kt == 0),
                                        stop=(kt == n_kt - 1))

                        for kt in range(n_kt):
                            k_sl = slice(kt * KT, (kt + 1) * KT)
                            eal = ealp.tile([128, H_PER_CORE, QT], bf16,
                                            tag="eal")
                            nc.sync.dma_start(
                                out=eal, in_=ealT[:, k_sl, q_sl].rearrange(
                                    "h p n -> p h n"))
                            exms = []
                            for b in range(B):
                                sc2 = sc2_ps.tile([128, H_PER_CORE, QT], f32,
                                                  tag="sc2", name="sc2")
                                for h in range(H_PER_CORE):
                                    c_sl = slice(h * 64, (h + 1) * 64)
                                    nc.tensor.matmul(
                                        sc2[:, h, :], kT_sb[c_sl, b, k_sl],
                                        qT_sb[c_sl, b, q_sl],
                                        start=True, stop=True,
                                        tile_position=(h * 64, 0))
                                ex2 = exp_sb.tile([128, H_PER_CORE, QT], bf16,
                                                  tag="ex2")
                                nc.scalar.activation(
                                    out=ex2, in_=sc2,
                                    func=mybir.ActivationFunctionType.Exp,
                                    bias=zeros128[:, 0:1], scale=1.0)
                                exm = exm_sb.tile([128, H_PER_CORE, QT], bf16,
                                                  tag="exm")
                                nc.vector.tensor_mul(exm, ex2, eal)
                                exms.append(exm)
                            pend.append((kt, exms))
                            # interleave previous qt's tail into the first
                            # kt iterations (cross-engine overlap; no PE
                            # head-of-line blocking)
                            if kt < len(tail_parts):
                                tail_parts[kt]()
                            if kt == 1:
                                # av alloc after the last readers (o_sb) of
                                # the previous qt's accumulators are emitted
                                av = [[av_ps.tile([65, QT], f32,
                                                  tag=f"av{b}{h}",
                                                  name=f"av{b}{h}")
                                       for h in range(H_PER_CORE)]
                                      for b in range(B)]
                            if len(pend) > 1 and av is not None:
                                flush_av(*pend.pop(0))
                        while pend:
                            flush_av(*pend.pop(0))
                        if qt + 1 < n_qt:
                            # PE filler at the qt boundary: keeps the PE
                            # p-state warm while Act/DVE drain the softmax
                            # tail of this qt
                            emit_qproj(qt + 1)
                        tail_parts = make_tail(qt, av)
                    for p in tail_parts:
                        p()
    nc.compile()
    return nc


_NC_CACHE = {}


def _get_program(n_tok=N_TOK, with_pbias=False):
    key = (n_tok, with_pbias)
    if key not in _NC_CACHE:
        _NC_CACHE[key] = build_program(n_tok, with_pbias)
    return _NC_CACHE[key]


def _prep_in_maps(x, context, alibi, Wq, Wk, Wv, Wo, bo, ln_w, ln_b):
    b, n, d = x.shape
    scale = (d // HEADS) ** -0.5

    x = np.asarray(x, dtype=np.float32)
    context = np.asarray(context, dtype=np.float32)
    alibi = np.asarray(alibi, dtype=np.float32)
    Wq, Wk, Wv, Wo = (np.asarray(w, dtype=np.float32) for w in (Wq, Wk, Wv, Wo))
    bo = np.asarray(bo, dtype=np.float32)
    ln_w = np.asarray(ln_w, dtype=np.float32)
    ln_b = np.asarray(ln_b, dtype=np.float32)

    xT = np.ascontiguousarray(x.transpose(0, 2, 1)).astype(BF16)
    cT = np.ascontiguousarray(context.transpose(0, 2, 1)).astype(BF16)

    in_maps = []
    with_pbias = False
    for ci in range(N_CORES):
        h0 = ci * H_PER_CORE
        cs = slice(h0 * DH, (h0 + H_PER_CORE) * DH)  # this core's 128 channels
        ealT = np.exp(np.ascontiguousarray(
            alibi[0, h0:h0 + H_PER_CORE].transpose(0, 2, 1))).astype(BF16)

        wq_s = (Wq[cs] * ln_w[None, :]) * scale          # [128, d]
        wk_s = Wk[cs] * ln_w[None, :]
        wv_s = Wv[cs] * ln_w[None, :]
        wbar = np.stack([
            -wq_s.sum(axis=1), -wk_s.sum(axis=1), -wv_s.sum(axis=1)])
        pb = np.stack([
            (Wq[cs] @ ln_b) * scale, Wk[cs] @ ln_b, Wv[cs] @ ln_b])  # [3,128]
        if np.abs(pb).max() > 0:
            with_pbias = True

        bo_core = bo if ci == 0 else np.zeros_like(bo)

        in_maps.append({
            "xT": xT,
            "cT": cT,
            "ealT": ealT,
            "wqT": np.ascontiguousarray(wq_s.T).astype(BF16),
            "wkT": np.ascontiguousarray(wk_s.T).astype(BF16),
            "wvT": np.ascontiguousarray(wv_s.T).astype(BF16),
            "wbar": wbar.astype(BF16),
            "woT": np.ascontiguousarray(Wo[:, cs].T).astype(BF16),
            "bo_r": np.ascontiguousarray(
                bo_core.reshape(N_DT, 128).T).astype(np.float32),
        })
        if with_pbias:
            in_maps[-1]["pbias"] = pb.astype(BF16)
    if with_pbias:
        for m in in_maps:
            m.setdefault("pbias", np.zeros((3, 128), dtype=BF16))
    return in_maps, with_pbias


def _gather(results, b, n, d):
    acc = np.zeros((d, b * n), dtype=np.float32)
    for r in results:
        acc += r["outT"].astype(np.float32)
    return np.ascontiguousarray(
        acc.reshape(d, b, n).transpose(1, 2, 0)).astype(np.float32)


def kernel(**inputs):
    from concourse.bass_utils import run_bass_kernel_spmd
    x = inputs["x"]
    b, n, d = x.shape
    in_maps, with_pbias = _prep_in_maps(**inputs)
    nc = _get_program(n, with_pbias)
    res = run_bass_kernel_spmd(nc, in_maps, list(range(N_CORES)))
    return _gather(res.results, b, n, d)


def run_profiled(inputs, trace=True):
    from concourse.bass_utils import run_bass_kernel_spmd
    x = inputs["x"]
    b, n, d = x.shape
    in_maps, with_pbias = _prep_in_maps(**inputs)
    nc = _get_program(n, with_pbias)
    res = run_bass_kernel_spmd(nc, in_maps, list(range(N_CORES)), trace=trace)
    return _gather(res.results, b, n, d), res
